# revision 2
# baseline (speedup 1.0000x reference)
"""Trainium2 Bass kernel for nn_DiscretisedBNF (histogram binning MLP).

Math: the reference's per-bin CDF sum telescopes exactly (kl_{k+1} == kr_k
bit-identically, and cdf(kl_0) = cdf(kr_0) = 0 since those bounds are <= -1),
so

    sum_k [cdf(kr_k) - cdf(kl_k)] = cdf(kr_{K-1}) = 0.5*(1 + erf((0.875-mu_x)*inv))

with mu_x = mu/gamma - s*mu_eps, inv = 1/(sigma_x*sqrt(2)), sigma_x =
s*exp(ln_sigma_eps), s = sqrt((1-gamma)/gamma).  Rearranged for the chip:

    arg = (A + mu_eps) * E
    A   = mu*qm + qa          qm = -1/(gamma*s), qa = 0.875/s   (per batch row)
    E   = exp(-ln_sigma_eps - ln(sqrt(2)))
    out = 0.5*erf(arg) + 0.5

Sharding: pure data parallel — batch dim (2048) split 256 rows per core;
weights replicated.

Precision plan (HW exec is DMA/PE balanced, so shrink both):
  - W1 stored fp8 e3m4 scaled x64 (values ~N(0,1) after scaling); the 1/64
    is folded into the Lrelu activation's input scale.
  - W2 stored fp8 e4m3 scaled x32; the 1/32 is folded into the Exp bias
    (ln 32) for the ln_sigma half and into qm/qa (x32) for the mu_eps half.
  - h stored fp8 e4m3 so matmul2 runs in DoubleRow perf mode (2 fp8
    contraction rows per PE cell -> ~2x matmul2 throughput).
  - Activations x^T stay fp16; epilogue mu fp16.
  Simulated end-to-end rel err of this config: 7.4e-3 (gate: 2e-2).

All inputs are SBUF-resident (~20 MB total), DRAM layouts partition-major so
every DMA moves multi-KB contiguous runs per partition.
"""

import numpy as np
import ml_dtypes
from contextlib import ExitStack

import concourse.bass as bass
import concourse.mybir as mybir
from concourse.tile import TileContext
from concourse.bass_utils import run_bass_kernel_spmd

B, D, H = 2048, 4096, 1024
NCORES = 8
BS = B // NCORES            # 256 batch rows per core
KC1 = (D + 1 + 127) // 128  # 33 contract chunks for matmul1 (D+1=4097 padded)
DPAD = KC1 * 128            # 4224
KC2 = H // 128              # 8 contract chunks for matmul2
NP2 = KC2 // 2              # 4 DoubleRow contraction pairs
NJ = D // 512               # 8 output column groups of 512
LEAKY_SLOPE = 0.01
LN_SQRT2 = 0.34657359027997264
LN_32 = 3.4657359027997265
SCALE_W1 = 64.0
SCALE_W2 = 32.0
USE_DR = True               # matmul2 DoubleRow fp8 perf mode

F16 = mybir.dt.float16
F32 = mybir.dt.float32
F8E3 = mybir.dt.float8e3
F8E4 = mybir.dt.float8e4
AF = mybir.ActivationFunctionType
OP = mybir.AluOpType
E3NP = ml_dtypes.float8_e3m4
E4NP = ml_dtypes.float8_e4m3

# front-loaded DMA split so matmul1 starts as early as possible
W1_PARTS = [1, 2, 3, 3, 4, 4, 4, 4, 4, 4]
XT_PARTS = [2, 6, 12, 13]


def split_multi_waits(nc):
    """This container's walrus accepts at most ONE sync-wait per instruction
    (setupSyncWait: 'Too many sync wait commands').  Split any instruction
    carrying N>1 waits into N-1 single-wait NoOps on the same engine placed
    immediately before it."""
    cnt = 0
    sync_info_cls = None
    for f in nc.m.functions:
        for bb in f.blocks:
            out = []
            changed = False
            for inst in bb.instructions:
                si = inst.sync_info
                waits = list(si.on_wait) if si and si.on_wait else []
                if len(waits) > 1:
                    if sync_info_cls is None:
                        sync_info_cls = type(si)
                    for w in waits[:-1]:
                        nop = mybir.InstNoOp(name=f"waitsplit_{cnt}", ins=[], outs=[])
                        cnt += 1
                        nop.engine = inst.engine
                        nop.sync_info = sync_info_cls(on_wait=[w], on_update=[])
                        out.append(nop)
                    si.on_wait = waits[-1:]
                    changed = True
                out.append(inst)
            if changed:
                bb.instructions = out
    return cnt


def _lean_drain_and_barrier(self, tick_clock, wait_clock):
    """Replacement for TileContext._drain_and_barrier: drain + ONE barrier,
    skipping the ~7us semaphore-clear butterfly.  The Bass preamble re-clears
    every kernel semaphore at the start of each execution, and no sibling
    TileContext follows this one, so the tail clear is redundant.  The
    multi-wait drain is split later by split_multi_waits."""
    import concourse.tile as tile_mod

    nc = self.nc
    drain_inst = nc.sync.drain()
    wait_clock.add_sem_waits(
        drain_inst.ins, tile_mod.ScopedClock({None: tick_clock.global_clock})
    )
    popped = nc._tile_sem_poison_stack.pop()
    assert popped is self._sem_poison


def _build():
    nc = bass.Bass()
    orig_drain = TileContext._drain_and_barrier
    TileContext._drain_and_barrier = _lean_drain_and_barrier
    try:
        _build_body(nc)
    finally:
        TileContext._drain_and_barrier = orig_drain

    split_multi_waits(nc)
    return nc


def _build_body(nc):
    # partition-major DRAM layouts: per partition, each load is one long
    # contiguous run
    xT = nc.dram_tensor("xT", [128, KC1, BS], F16, kind="ExternalInput")
    w1 = nc.dram_tensor("w1", [128, KC1, H], F8E3, kind="ExternalInput")
    w2 = nc.dram_tensor("w2", [128, NJ, KC2, 2, 512], F8E4, kind="ExternalInput")
    b1c = nc.dram_tensor("b1c", [128, KC2], F32, kind="ExternalInput")
    b2c = nc.dram_tensor("b2c", [1, 2 * D], F16, kind="ExternalInput")
    mun = nc.dram_tensor("mun", [128, 2, NJ, 512], F16, kind="ExternalInput")
    qmd = nc.dram_tensor("qm", [128, 2], F32, kind="ExternalInput")
    qad = nc.dram_tensor("qa", [128, 2], F32, kind="ExternalInput")
    outd = nc.dram_tensor("out", [128, 2, NJ, 512], F16, kind="ExternalOutput")

    with TileContext(nc) as tc, ExitStack() as ctx:
        const = ctx.enter_context(tc.tile_pool(name="const", bufs=1))
        res = ctx.enter_context(tc.tile_pool(name="res", bufs=1))
        hpool = ctx.enter_context(tc.tile_pool(name="hpool", bufs=1))
        eppool = ctx.enter_context(tc.tile_pool(name="eppool", bufs=4))
        outpool = ctx.enter_context(tc.tile_pool(name="outpool", bufs=3))
        pspool = ctx.enter_context(tc.tile_pool(name="pspool", bufs=8, space="PSUM"))

        # --- constants (no-DMA first: feed the PE warm-up burst) ---
        ones_sb = const.tile([1, 128], F16, name="ones_sb")
        nc.vector.memset(ones_sb[:], 1.0)
        ones_row = const.tile([128, 256], F16, name="ones_row")
        nc.vector.memset(ones_row[:], 1.0)
        ones128 = const.tile([128, 128], F16, name="ones128")
        nc.vector.memset(ones128[:], 1.0)
        nln_sb = const.tile([128, 1], F32, name="nln_sb")
        nc.vector.memset(nln_sb[:], -(LN_SQRT2 + LN_32))

        # PE warm-up: dependency-free full-rank matmuls so the HAM clock
        # gate opens; just long enough to cover the first W1/xT DMA parts.
        ps_warm = pspool.tile([128, 512], F32, tag="ps", name="ps_warm")
        for _ in range(14):
            nc.tensor.matmul(
                ps_warm[:, :BS], ones128[:], ones_row[:], start=True, stop=True
            )

        # tiny const loads on the SWDGE ring so the HWDGE rings' FIFO heads
        # belong to the W1/xT streams.
        b1_sb = const.tile([128, KC2], F32, name="b1_sb")
        nc.gpsimd.dma_start(out=b1_sb[:], in_=b1c[:])
        b2_sb = const.tile([1, 2 * D], F16, name="b2_sb")
        nc.gpsimd.dma_start(out=b2_sb[:], in_=b2c[:])
        qm_sb = const.tile([128, 2], F32, name="qm_sb")
        nc.gpsimd.dma_start(out=qm_sb[:], in_=qmd[:])
        qa_sb = const.tile([128, 2], F32, name="qa_sb")
        nc.gpsimd.dma_start(out=qa_sb[:], in_=qad[:])

        # --- resident activation x^T (contract dim on partitions); Scalar
        # HWDGE ring, front-loaded small parts so mm1 starts early.
        xt_tiles = {}
        k0 = 0
        for q, nk in enumerate(XT_PARTS):
            xt_q = res.tile([128, nk, BS], F16, tag=f"xt{q}", name=f"xt_q{q}")
            nc.scalar.dma_start(out=xt_q[:], in_=xT[:, k0 : k0 + nk, :])
            for i in range(nk):
                xt_tiles[k0 + i] = xt_q[:, i, :]
            k0 += nk
        assert k0 == KC1

        # epilogue mu, fully resident (needed only from the first epilogue)
        mu_sb = res.tile([128, 2, NJ, 512], F16, tag="mu", name="mu_sb")
        nc.scalar.dma_start(out=mu_sb[:], in_=mun[:])

        # --- matmul1: h^T = W1^T @ x^T, H on partitions (8 psum tiles) ---
        ps1 = [
            pspool.tile([128, 512], F32, tag="ps", name=f"ps1_{m}")[:, :BS]
            for m in range(KC2)
        ]
        w1_tiles = {}
        k0 = 0
        for g, nk in enumerate(W1_PARTS):
            w1g = res.tile([128, nk, H], F8E3, tag=f"w1g{g}", name=f"w1g{g}")
            nc.sync.dma_start(out=w1g[:], in_=w1[:, k0 : k0 + nk, :])
            for i in range(nk):
                w1_tiles[k0 + i] = w1g[:, i, :]
            k0 += nk
        assert k0 == KC1

        # W2 per-j blocks on the Sync ring behind the W1 stream; all resident.
        w2_tiles = []
        for j in range(NJ):
            w2t = res.tile([128, KC2, 2, 512], F8E4, tag=f"w2_{j}", name=f"w2t{j}")
            nc.sync.dma_start(out=w2t[:], in_=w2[:, j])
            w2_tiles.append(w2t)

        for k in range(KC1):
            for m in range(KC2):
                nc.tensor.matmul(
                    ps1[m],
                    w1_tiles[k][:, m * 128 : (m + 1) * 128],
                    xt_tiles[k],
                    start=(k == 0),
                    stop=(k == KC1 - 1),
                )

        # h -> fp8 e4m3, stored as DoubleRow pairs: tile p holds H-chunks
        # (2p, 2p+1); the Lrelu activation folds in bias and the 1/64 W1
        # descale.
        h8 = []
        for p in range(NP2):
            hp = hpool.tile([128, 2, BS], F8E4, tag=f"h{p}", name=f"h8_{p}")
            for i in range(2):
                m = 2 * p + i
                nc.scalar.activation(
                    hp[:, i, :],
                    ps1[m],
                    AF.Lrelu,
                    bias=b1_sb[:, m : m + 1],
                    scale=1.0 / SCALE_W1,
                    alpha=LEAKY_SLOPE,
                )
            h8.append(hp)

        # --- matmul2 (DoubleRow fp8) + fused epilogue, batch on partitions ---
        for j in range(NJ):
            w2t = w2_tiles[j]
            psA = [
                pspool.tile([128, 512], F32, tag="ps", name=f"psA{j}_{bh}")
                for bh in range(2)
            ]
            psB = [
                pspool.tile([128, 512], F32, tag="ps", name=f"psB{j}_{bh}")
                for bh in range(2)
            ]
            # seed PSUM with the (x32-scaled) b2 bias row via rank-1 matmul
            for bh in range(2):
                nc.tensor.matmul(
                    psA[bh][:],
                    ones_sb[:],
                    b2_sb[:, j * 512 : (j + 1) * 512],
                    start=True,
                    stop=False,
                )
                nc.tensor.matmul(
                    psB[bh][:],
                    ones_sb[:],
                    b2_sb[:, D + j * 512 : D + (j + 1) * 512],
                    start=True,
                    stop=False,
                )
            if USE_DR:
                for p in range(NP2):
                    for bh in range(2):
                        lhs = h8[p][:, :, bh * 128 : (bh + 1) * 128]
                        nc.tensor.matmul(
                            psA[bh][:],
                            lhs,
                            w2t[:, 2 * p : 2 * p + 2, 0, :],
                            start=False,
                            stop=(p == NP2 - 1),
                            perf_mode=mybir.MatmulPerfMode.DoubleRow,
                        )
                        nc.tensor.matmul(
                            psB[bh][:],
                            lhs,
                            w2t[:, 2 * p : 2 * p + 2, 1, :],
                            start=False,
                            stop=(p == NP2 - 1),
                            perf_mode=mybir.MatmulPerfMode.DoubleRow,
                        )
            else:
                for k in range(KC2):
                    for bh in range(2):
                        lhs = h8[k // 2][:, k % 2, bh * 128 : (bh + 1) * 128]
                        nc.tensor.matmul(
                            psA[bh][:], lhs, w2t[:, k, 0, :],
                            start=False, stop=(k == KC2 - 1),
                        )
                        nc.tensor.matmul(
                            psB[bh][:], lhs, w2t[:, k, 1, :],
                            start=False, stop=(k == KC2 - 1),
                        )

            o2 = outpool.tile([128, 2, 512], F16, tag="o", name=f"O{j}")
            # consume all four PSUM tiles first (EXP reads psB on ACT, the
            # add reads psA on DVE) so the banks release early for j+2
            e2s, s2s = [], []
            for bh in range(2):
                e2 = eppool.tile([128, 512], F32, tag="E", name=f"E{j}_{bh}")
                nc.scalar.activation(
                    e2[:], psB[bh][:], AF.Exp, bias=nln_sb[:], scale=-1.0 / SCALE_W2
                )
                e2s.append(e2)
            for bh in range(2):
                a2 = eppool.tile([128, 512], F32, tag="A", name=f"A{j}_{bh}")
                nc.vector.tensor_scalar(
                    a2[:],
                    mu_sb[:, bh, j, :],
                    qm_sb[:, bh : bh + 1],
                    qa_sb[:, bh : bh + 1],
                    OP.mult,
                    OP.add,
                )
                s2 = eppool.tile([128, 512], F32, tag="S", name=f"S{j}_{bh}")
                nc.vector.tensor_tensor(s2[:], psA[bh][:], a2[:], OP.add)
                s2s.append(s2)
            for bh in range(2):
                g2 = eppool.tile([128, 512], F32, tag="G", name=f"G{j}_{bh}")
                nc.vector.tensor_tensor(g2[:], s2s[bh][:], e2s[bh][:], OP.mult)
                r2 = eppool.tile([128, 512], F32, tag="R", name=f"R{j}_{bh}")
                nc.scalar.activation(r2[:], g2[:], AF.Erf)
                nc.vector.tensor_scalar(
                    o2[:, bh, :], r2[:], 0.5, 0.5, OP.mult, OP.add
                )
            nc.gpsimd.dma_start(out=outd[:, :, j, :], in_=o2[:])


_NC = None
_last_in_maps = None


def kernel(mu, t, gamma, W1, b1, W2, b2):
    global _NC
    if _NC is None:
        _NC = _build()
    nc = _NC

    f16 = np.float16
    f32 = np.float32

    # x^T = concat([mu, t], 1)^T, zero-padded to DPAD rows, fp16,
    # partition-major [128, KC1, B]
    Xt = np.zeros((DPAD, B), dtype=f16)
    Xt[:D, :] = np.asarray(mu, dtype=f32).T
    Xt[D, :] = np.asarray(t, dtype=f32)[:, 0]
    Xt_pm = np.ascontiguousarray(Xt.reshape(KC1, 128, B).transpose(1, 0, 2))

    W1p = np.zeros((DPAD, H), dtype=f32)
    W1p[: D + 1, :] = np.asarray(W1, dtype=f32) * np.float32(SCALE_W1)
    w1_np = np.ascontiguousarray(
        W1p.reshape(KC1, 128, H).transpose(1, 0, 2)
    ).astype(E3NP)

    # [ki, j, k, h, n] = 32*W2[k*128+ki, h*D + j*512 + n]
    W2s = (np.asarray(W2, dtype=f32) * np.float32(SCALE_W2)).reshape(
        KC2, 128, 2, NJ, 512
    )
    w2_np = np.ascontiguousarray(W2s.transpose(1, 3, 0, 2, 4)).astype(E4NP)

    b1c_np = np.ascontiguousarray(np.asarray(b1, dtype=f32).reshape(KC2, 128).T)
    b2c_np = (np.asarray(b2, dtype=f32) * np.float32(SCALE_W2)).astype(f16).reshape(
        1, 2 * D
    )

    g64 = np.asarray(gamma, dtype=np.float64)[:, 0]
    s64 = np.sqrt((1.0 - g64) / g64)
    qm_full = (-SCALE_W2 / (g64 * s64)).astype(f32)
    qa_full = (SCALE_W2 * 0.875 / s64).astype(f32)
    mu16 = np.asarray(mu, dtype=f32).astype(f16)

    in_maps = []
    for c in range(NCORES):
        sl = slice(c * BS, (c + 1) * BS)
        in_maps.append(
            {
                "xT": np.ascontiguousarray(Xt_pm[:, :, sl]),
                "w1": w1_np,
                "w2": w2_np,
                "b1c": b1c_np,
                "b2c": b2c_np,
                "mun": np.ascontiguousarray(
                    mu16[sl].reshape(2, 128, NJ, 512).transpose(1, 0, 2, 3)
                ),
                "qm": np.ascontiguousarray(qm_full[sl].reshape(2, 128).T),
                "qa": np.ascontiguousarray(qa_full[sl].reshape(2, 128).T),
            }
        )

    global _last_in_maps
    _last_in_maps = in_maps

    res = run_bass_kernel_spmd(nc, in_maps, core_ids=list(range(NCORES)))
    # out [128, 2, NJ, 512] -> [BS, D] (batch row b = bh*128 + p)
    return np.concatenate(
        [
            r["out"].astype(np.float32).transpose(1, 0, 2, 3).reshape(BS, D)
            for r in res.results
        ],
        axis=0,
    )
